# revision 43
# baseline (speedup 1.0000x reference)
"""Fused bidirectional (ESIM) attention kernel for Trainium2 (Bass/Tile).

Problem: B=16, Lp=Lh=2048, D=256 fp32.
  sim = P @ H^T / sqrt(D)
  attended_premises   = masked_softmax(sim,   hm) @ H * pm
  attended_hypotheses = masked_softmax(sim^T, pm) @ P * hm

Key identities / tricks:
  - softmax(scores*mask)*mask / (sum + EPS) reduces to
    out_j = e_j*m_j / sum_k e_k*m_k with e = exp(scores).
  - Masks are dense 0/1 with p=0.5: the host COMPACTS each sequence to its
    valid rows (padded to Lv = max valid count rounded up to 128). All
    device matmuls run on ~Lv=1152 instead of 2048 rows per side (~3.2x
    less PE work; PE is the bottleneck). Outputs scatter back on the host.
  - bf16 operands (fp32 PSUM accumulation): same PE rate as float32r but
    half the DMA traffic; rel-err ~5e-3, inside the 2e-2 gate.
  - Scores are computed ONCE (orientation [h, p]). The second direction's
    E^T comes from dma_start_transpose (DMA xbar, SBUF->SBUF, block-major
    fold [128, G*128] -> [128, G, 128] verified on HW), saving 25% of PE
    work vs recomputing the scores transposed. E chunks are grouped 3 per
    tile so one transpose instruction covers 3 chunks (the per-instruction
    queue cost is what hurts).
  - Both directions' softmax denominators ride the weighted-sum matmuls
    via two valid-indicator columns LEADING the compacted K-side rows
    (N=258, layout [1 1 | data]); no separate reduction anywhere. Leading
    placement lets the final tile's split accumulation normalize early.
  - Outputs are written bf16 (computed in fp32, rounded on store; host
    converts back) — halves output DMA and leaves the sync ring ~10us of
    slack (it is otherwise ~saturated at its ~196 GB/s effective rate).
  - DMA queue discipline: the SP queue pays ~5ns per descriptor and each
    dma_start costs ~0.6us of sequencer issue (128 descriptors), so loads
    are 2 fat DMAs per batch and outputs are staged in SBUF and written as
    3-chunk contiguous DMAs. Fine-grained region-split loads backfire:
    they inflate the NEFF semaphore-init phase (~+0.4us per extra region
    DMA when many regions interleave).
  - A PE warmup burst of dummy matmuls runs during the ~8us DMA/queue
    startup so real matmuls start at the HAM-warm 2.4 GHz clock (the ramp
    needs ~5.6us of sustained PE activity and sags within ~2us of idle).
  - The last weighted-sum tile's serial recip->mul->dma tail (~1.7us) is
    shortened by accumulating it in two PSUM tiles (dep tracking is tile-
    granular): denominator + 160 data cols first, so the reciprocal and
    first chunk's normalize+DMA hide under the 96-col remainder's matmuls;
    final-direction drains alternate the scalar/sync queues.

Sharding: data-parallel over batch, 2 batches per core on 8 cores.

Per batch (Lv padded valid length, NC = Lv/128 chunks, GC = NC/3 groups):
  A1: V1[h,p] per 128-row h-chunk; exp -> E1 chunk (bf16, grouped 3/tile);
      per group one dma-transpose into the E2 buffer [128, 27, 128].
  B1: per p-tile: acc[p, :258] = sum_h E1ᵀ @ [1 1 | Hc]; out = acc[:, 2:]
      * 1/acc[:, 0] into a staging tile, 3-chunk DMAs out.
  B2: same with E2 group slices and [1 1 | Pc].
Emission order A1(0) B1(0) A1(1) B2(0) B1(1) B2(1) keeps the PE FIFO from
head-of-line blocking on batch-0 transposes.
"""

import numpy as np
import ml_dtypes

import concourse.mybir as mybir
import concourse.tile as tile
from concourse import bacc
from concourse.bass_utils import run_bass_kernel_spmd

F32 = mybir.dt.float32
BF16 = mybir.dt.bfloat16
EXP = mybir.ActivationFunctionType.Exp
BF16NP = ml_dtypes.bfloat16

B, L, D = 16, 2048, 256
NCORES = 8
BPC = B // NCORES      # batches per core
DC = D // 128          # 2 contraction chunks of 128 for the score matmuls
NAUG = D + 2           # compacted K-side rows + two valid-indicator columns
GRP = 3                # E chunks per transpose group
SCALE = 1.0 / np.sqrt(np.float32(D)).astype(np.float32)


def PSW(Lv):
    """Score-PSUM tile width, rounded to a whole PSUM bank multiple."""
    return ((Lv * 4 + 2047) // 2048) * 512


def _scores(nc, ldA, Lv, Nq, psv, ep, e2p, tag, soft=False):
    """Single score pass: V1[h,p] per h-chunk, exp into grouped E tiles,
    one dma-transpose per group into the E2 chunk-folded layout.

    Score matmuls only cover query columns [0, Nq) (the max valid premise
    count): E columns beyond hold exp(stale PSUM), which is bounded (PSUM
    score banks only ever contain zeros from warmup or old score values),
    and they only feed dropped output partitions / zeroed contractions.

    Returns (E1 group tiles, E2 group tiles)."""
    NC = Lv // 128
    GC = NC // GRP
    psw = PSW(Lv)
    la0, la1 = ldA
    xth = [la0[:, 0:Lv], la1[:, 0:Lv]]
    xtp = [la0[:, Lv : 2 * Lv], la1[:, Lv : 2 * Lv]]
    e1g, e2g = [], []
    for g in range(GC):
        e1g.append(ep.tile([128, GRP, Lv], BF16, tag=f"E{g}", name=f"E{tag}_{g}"))
        e2g.append(
            e2p.tile([128, GRP * NC, 128], BF16, tag=f"T{g}", name=f"T{tag}_{g}")
        )
    def mm_dc(ps, kc, dc):
        for off in range(0, Nq, 512):
            w = min(512, Nq - off)
            nc.tensor.matmul(
                ps[:, off : off + w],
                lhsT=xth[dc][:, kc * 128 : (kc + 1) * 128],
                rhs=xtp[dc][:, off : off + w],
                start=(dc == 0),
                stop=(dc == DC - 1),
            )

    def finish(ps, kc):
        g, r = divmod(kc, GRP)
        nc.scalar.activation(e1g[g][:, r, :], ps[:, 0:Lv], EXP, scale=float(SCALE))
        if r == GRP - 1:
            nc.sync.dma_start_transpose(e2g[g][:, :, :], e1g[g][:, :, :])

    kc0 = 0
    if soft and NC >= 2 and DC == 2:
        # Kernel-start softening: the dc=1 operands (second load half) land
        # ~2us after dc=0's. Run BOTH first chunks' dc=0 sweeps (two open
        # PSUM accumulations = both psv buffers) before touching dc=1 so
        # the PE has work while that transfer finishes.
        ps0 = psv.tile([128, psw], F32, tag="v", name=f"v{tag}0")
        ps1 = psv.tile([128, psw], F32, tag="v", name=f"v{tag}1")
        mm_dc(ps0, 0, 0)
        mm_dc(ps1, 1, 0)
        mm_dc(ps0, 0, 1)
        finish(ps0, 0)
        mm_dc(ps1, 1, 1)
        finish(ps1, 1)
        kc0 = 2
    for kc in range(kc0, NC):
        ps = psv.tile([128, psw], F32, tag="v", name=f"v{tag}{kc}")
        for dc in range(DC):
            mm_dc(ps, kc, dc)
        finish(ps, kc)
    return e1g, e2g


def _wsum(nc, lhs_of, xa, out_dram, Lv, pac, den, outp, tag, final=False,
          soft=False, psv=None):
    """One direction's weighted sum + normalize + staged output DMAs.

    lhs_of(qt, kc) -> [128, 128] bf16 lhsT slice (contraction chunk kc for
    query tile qt). xa: [128, NC*NAUG] K-side rows with valid columns.
    Outputs stage in SBUF and leave as 3-chunk DMAs on the Sync queue;
    final=True instead streams per-tile from the (by then idle) ACT hwdge
    queue so the kernel tail is one 128-row transfer.
    soft=True interleaves the first two query tiles' accumulations (both
    PSUM accumulator buffers open) so their last-group matmuls don't catch
    up with the score pass's final exp (whole-tile dependency)."""
    NC = Lv // 128
    accs = {}

    def mm_run(qt, k0, k1, c0=0, c1=NAUG):
        for kc in range(k0, k1):
            nc.tensor.matmul(
                accs[qt][:, c0:c1],
                lhsT=lhs_of(qt, kc),
                rhs=xa[:, kc * NAUG + c0 : kc * NAUG + c1],
                start=(kc == 0),
                stop=(kc == NC - 1),
            )

    def finish(qt, stage, qr):
        # Augmented-row layout: cols [0:2) valid indicators (denominator),
        # data at [2:NAUG).
        r = den.tile([128, 1], F32, tag="rec", name=f"rec{tag}_{qt}")
        nc.vector.reciprocal(r[:], accs[qt][:, 0:1])
        nc.vector.tensor_scalar_mul(
            stage[:, qr * D : (qr + 1) * D], accs[qt][:, 2 : 2 + D], r[:]
        )
        if final:
            # Alternate per-tile drain DMAs over two queues: a dma_start
            # costs ~0.6us of sequencer issue time (128 descriptors), so on
            # one queue the last tile's write would serialize behind eight
            # earlier issues.
            eng = (nc.scalar, nc.sync)[qt % 2]
            eng.dma_start(
                out=out_dram[:, qt * D : (qt + 1) * D],
                in_=stage[:, qr * D : (qr + 1) * D],
            )

    def finish_tail(qt, stage, qr, psv):
        # The very last tile gates the kernel end with a serial
        # recip->mul->dma chain (~1.7us). Its accumulation runs as two
        # column groups in SEPARATE PSUM tiles (dep tracking is tile-
        # granular, so one shared tile would serialize the reciprocal
        # behind both chains) — the first group holds the denominator
        # plus most of the data, so the reciprocal and the first chunk's
        # normalize+DMA hide under the second group's matmuls; only a
        # 96-col remainder drains after the last matmul.
        da = 160
        ca = 2 + da
        accb = pac.tile([128, 512], F32, tag="acc", name=f"accb{tag}")
        mm_run(qt, 0, NC, 0, ca)
        for kc in range(NC):
            nc.tensor.matmul(
                accb[:, 0 : NAUG - ca],
                lhsT=lhs_of(qt, kc),
                rhs=xa[:, kc * NAUG + ca : (kc + 1) * NAUG],
                start=(kc == 0),
                stop=(kc == NC - 1),
            )
        r = den.tile([128, 1], F32, tag="rec", name=f"rec{tag}_{qt}")
        nc.vector.reciprocal(r[:], accs[qt][:, 0:1])
        nc.vector.tensor_scalar_mul(
            stage[:, qr * D : qr * D + da], accs[qt][:, 2:ca], r[:]
        )
        nc.scalar.dma_start(
            out=out_dram[:, qt * D : qt * D + da],
            in_=stage[:, qr * D : qr * D + da],
        )
        nc.vector.tensor_scalar_mul(
            stage[:, qr * D + da : (qr + 1) * D], accb[:, 0 : NAUG - ca], r[:]
        )
        nc.sync.dma_start(
            out=out_dram[:, qt * D + da : (qt + 1) * D],
            in_=stage[:, qr * D + da : (qr + 1) * D],
        )

    for g in range(NC // GRP):
        stage = outp.tile(
            [128, GRP * D], BF16, tag=f"st{tag[0]}{g}", name=f"st{tag}_{g}"
        )
        for qr in range(GRP):
            qt = g * GRP + qr
            if soft and NC > GRP and g == 0 and qr in (0, 1):
                if qr == 0:
                    # qt0 up to the last chunk group, then qt1 likewise,
                    # then both tails — by which time the final exp landed.
                    accs[0] = pac.tile(
                        [128, 512], F32, tag="acc", name=f"acc{tag}_0"
                    )
                    accs[1] = pac.tile(
                        [128, 512], F32, tag="acc", name=f"acc{tag}_1"
                    )
                    mm_run(0, 0, NC - GRP)
                    mm_run(1, 0, NC - GRP)
                    mm_run(0, NC - GRP, NC)
                    finish(0, stage, 0)
                else:
                    mm_run(1, NC - GRP, NC)
                    finish(1, stage, 1)
                continue
            accs[qt] = pac.tile([128, 512], F32, tag="acc", name=f"acc{tag}_{qt}")
            if final and qt == NC - 1:
                finish_tail(qt, stage, qr, psv)
            else:
                mm_run(qt, 0, NC)
                finish(qt, stage, qr)
        if not final:
            nc.sync.dma_start(
                out=out_dram[:, g * GRP * D : (g + 1) * GRP * D],
                in_=stage[:],
            )


def build_program(Lv, Nq, bpc=BPC):
    NC = Lv // 128
    assert NC % GRP == 0
    # The tuned pipeline (double buffering everywhere) is sized for the
    # expected ~50% masks (Lv=1152). Denser masks still fit on-device up to
    # Lv=1920 with single buffering; beyond that run() falls back to host.
    sb = 2 if Lv <= 1152 else 1
    pb = 2 if 2 * (PSW(Lv) // 512) + 2 <= 8 else 1
    nc = bacc.Bacc("TRN2", target_bir_lowering=False, debug=False, num_devices=NCORES)
    # ldA: the four d-major score operands [xth0|xtp0|xth1|xtp1].
    # ldB: the two augmented K-side row blocks [xAh|xAp].
    ldA_t = nc.dram_tensor("ldA", [bpc, 128, 4 * Lv], BF16, kind="ExternalInput").ap()
    ldB_t = nc.dram_tensor(
        "ldB", [bpc, 128, 2 * NC * NAUG], BF16, kind="ExternalInput"
    ).ap()
    out_p = nc.dram_tensor("out_prem", [bpc, 128, NC * D], BF16, kind="ExternalOutput").ap()
    out_h = nc.dram_tensor("out_hyp", [bpc, 128, NC * D], BF16, kind="ExternalOutput").ap()

    with tile.TileContext(nc) as tc:
        with (
            tc.tile_pool(name="tp", bufs=sb) as tp,
            tc.tile_pool(name="ep", bufs=sb) as ep,
            tc.tile_pool(name="e2p", bufs=sb) as e2p,
            tc.tile_pool(name="outp", bufs=sb) as outp,
            tc.tile_pool(name="psv", bufs=pb, space="PSUM") as psv,
            tc.tile_pool(name="pac", bufs=2, space="PSUM") as pac,
            tc.tile_pool(name="den", bufs=4) as den,
        ):
            # PE warmup: queue bring-up + the first operand DMA keep the PE
            # data-starved until ~10.7us; dummy matmuls meanwhile ramp the
            # HAM clock so real matmuls start nearer 2.4 GHz. The dummies
            # also sweep every (buffer, bank) region of the score-PSUM
            # pool, zeroing whatever a previous NEFF left there (exp of a
            # stale inf in the skipped pad columns would ride the transpose
            # into direction-2 lhsT rows and NaN valid outputs via inf*0).
            wuw = tp.tile([128, 128], BF16, tag="wuw", name="wuw")
            nc.vector.memset(wuw[:], 0)
            wur = tp.tile([128, 512], BF16, tag="wur", name="wur")
            nc.vector.memset(wur[:], 0)
            nreg = PSW(Lv) // 512
            for i in range(9):
                pswu = psv.tile([128, PSW(Lv)], F32, tag="v", name=f"wu{i}")
                off = ((i // 2) % nreg) * 512
                nc.tensor.matmul(
                    pswu[:, off : off + 512], lhsT=wuw[:], rhs=wur[:],
                    start=True, stop=True,
                )

            def loads(b, fine=False):
                # Split the score operands in two so the dc=0 pair lands
                # ~1.5us earlier than one fat transfer would. For the head-
                # critical first batch, la0 splits further (lhsT half, then
                # the rhs in sweep-sized pieces) so the first score sweep
                # starts ~0.5us before the full la0 would have landed.
                # Aggressive fine-splitting backfires: many-region DMA
                # patterns inflate the NEFF's semaphore-init phase
                # (measured +2.9us for 7 extra region DMAs).
                la0 = tp.tile([128, 2 * Lv], BF16, tag="ldA0", name=f"ldA0_{b}")
                if fine and Lv > 512:
                    nc.sync.dma_start(out=la0[:, 0:Lv], in_=ldA_t[b, :, 0:Lv])
                    nc.sync.dma_start(
                        out=la0[:, Lv : Lv + 512], in_=ldA_t[b, :, Lv : Lv + 512]
                    )
                    nc.sync.dma_start(
                        out=la0[:, Lv + 512 : 2 * Lv],
                        in_=ldA_t[b, :, Lv + 512 : 2 * Lv],
                    )
                else:
                    nc.sync.dma_start(out=la0[:], in_=ldA_t[b, :, 0 : 2 * Lv])
                la1 = tp.tile([128, 2 * Lv], BF16, tag="ldA1", name=f"ldA1_{b}")
                nc.sync.dma_start(out=la1[:], in_=ldA_t[b, :, 2 * Lv : 4 * Lv])
                lb = tp.tile([128, 2 * NC * NAUG], BF16, tag="ldB", name=f"ldB{b}")
                nc.sync.dma_start(out=lb[:], in_=ldB_t[b])
                return (la0, la1), lb

            def d1_lhs(e1g):
                def f(qt, kc):
                    return e1g[kc // GRP][:, kc % GRP, qt * 128 : (qt + 1) * 128]
                return f

            def d2_lhs(e2g):
                def f(qt, kc):
                    # e2 group tile j-index = (h-chunk within group)*NC + pc
                    return e2g[qt // GRP][:, (qt % GRP) * NC + kc, :]
                return f

            st = [None] * bpc
            eg = [None] * bpc
            st[0] = loads(0)
            eg[0] = _scores(nc, st[0][0], Lv, Nq, psv, ep, e2p, "0", soft=True)
            _wsum(
                nc, d1_lhs(eg[0][0]), st[0][1][:, 0 : NC * NAUG], out_p[0],
                Lv, pac, den, outp, "p0", soft=True,
            )
            for b in range(bpc):
                if b + 1 < bpc:
                    st[b + 1] = loads(b + 1)
                    eg[b + 1] = _scores(
                        nc, st[b + 1][0], Lv, Nq, psv, ep, e2p, f"{b+1}"
                    )
                _wsum(
                    nc, d2_lhs(eg[b][1]), st[b][1][:, NC * NAUG :], out_h[b],
                    Lv, pac, den, outp, f"h{b}", final=(b == bpc - 1), psv=psv,
                )
                if b + 1 < bpc:
                    _wsum(
                        nc, d1_lhs(eg[b + 1][0]), st[b + 1][1][:, 0 : NC * NAUG],
                        out_p[b + 1], Lv, pac, den, outp, f"p{b+1}", soft=True,
                    )
    nc.compile()
    return nc


_PROGRAMS = {}


def _get_program(Lv, Nq):
    if (Lv, Nq) not in _PROGRAMS:
        _PROGRAMS[(Lv, Nq)] = build_program(Lv, Nq)
    return _PROGRAMS[(Lv, Nq)]


def _prep_side(x, idx, n, Lv):
    """Compact one (batch, side) to its valid rows.

    Returns (xT [128, 2*Lv] d-major halves, xA [128, NC*NAUG] chunk-row
    layout with two LEADING valid-indicator columns — the denominator rides
    in front so the final tile's split accumulation can normalize its first
    half early)."""
    NC = Lv // 128
    xc = np.zeros((Lv, NAUG), np.float32)
    xc[:n, 0:2] = 1.0
    xc[:n, 2:] = x[idx]
    t = xc[:, 2:].reshape(Lv, DC, 128).transpose(1, 2, 0)  # [dc, d, row]
    xA = np.ascontiguousarray(
        xc.reshape(NC, 128, NAUG).transpose(1, 0, 2).reshape(128, NC * NAUG)
    )
    return t, xA


def _host_fallback(pb, hb, pm, hm):
    """Exact reference semantics in numpy for inputs the device path can't
    hold (mask density far above the expected ~50%)."""
    def msoft(s, m):
        s = s * m
        s = s - s.max(axis=-1, keepdims=True)
        e = np.exp(s) * m
        return e / (e.sum(axis=-1, keepdims=True) + 1e-13)

    out_p = np.empty(pb.shape, np.float32)
    out_h = np.empty(hb.shape, np.float32)
    scale = 1.0 / np.sqrt(pb.shape[-1])
    for b in range(pb.shape[0]):
        sim = (pb[b].astype(np.float64) @ hb[b].T.astype(np.float64)) * scale
        pmb = pm[b].astype(np.float64)
        hmb = hm[b].astype(np.float64)
        out_p[b] = (msoft(sim, hmb[None, :]) @ hb[b].astype(np.float64)) * pmb[:, None]
        out_h[b] = (msoft(sim.T, pmb[None, :]) @ pb[b].astype(np.float64)) * hmb[:, None]
    return out_p, out_h


def run(premise_batch, premise_mask, hypothesis_batch, hypothesis_mask, trace=False):
    pb = np.asarray(premise_batch, dtype=np.float32)
    hb = np.asarray(hypothesis_batch, dtype=np.float32)
    pm = np.asarray(premise_mask)
    hm = np.asarray(hypothesis_mask)

    idx_p = [np.flatnonzero(pm[b]) for b in range(B)]
    idx_h = [np.flatnonzero(hm[b]) for b in range(B)]
    n_p = [len(i) for i in idx_p]
    n_h = [len(i) for i in idx_h]
    n_max = max(max(n_p), max(n_h), 128)
    Lv = ((n_max + 383) // 384) * 384  # multiple of 384 so NC % GRP == 0
    if Lv > 1920:
        # Near-dense masks don't fit the compacted on-device pipeline;
        # fall back to a correct host computation (not the perf path).
        return _host_fallback(pb, hb, pm, hm), None
    Nq = min(((max(max(n_p), 1) + 15) // 16) * 16, Lv)  # score query extent
    nc = _get_program(Lv, Nq)

    NC = Lv // 128
    ldA = np.empty((B, 128, 4 * Lv), BF16NP)
    ldB = np.empty((B, 128, 2 * NC * NAUG), BF16NP)
    for b in range(B):
        tp_, xAp = _prep_side(pb[b], idx_p[b], n_p[b], Lv)
        th_, xAh = _prep_side(hb[b], idx_h[b], n_h[b], Lv)
        ldA[b, :, 0 * Lv : 1 * Lv] = th_[0]
        ldA[b, :, 1 * Lv : 2 * Lv] = tp_[0]
        ldA[b, :, 2 * Lv : 3 * Lv] = th_[1]
        ldA[b, :, 3 * Lv : 4 * Lv] = tp_[1]
        ldB[b, :, 0 : NC * NAUG] = xAh
        ldB[b, :, NC * NAUG :] = xAp

    in_maps = []
    for c in range(NCORES):
        s = slice(c * BPC, (c + 1) * BPC)
        in_maps.append({"ldA": ldA[s], "ldB": ldB[s]})
    res = None
    for attempt in range(3):
        try:
            res = run_bass_kernel_spmd(nc, in_maps, list(range(NCORES)), trace=trace)
            break
        except Exception:
            # Transient device wedges (NRT_EXEC_UNIT_UNRECOVERABLE etc.)
            # usually clear on re-execution.
            if attempt == 2:
                raise
    ocp = np.concatenate([res.results[c]["out_prem"] for c in range(NCORES)], axis=0)
    och = np.concatenate([res.results[c]["out_hyp"] for c in range(NCORES)], axis=0)
    out_p = np.zeros((B, L, D), np.float32)
    out_h = np.zeros((B, L, D), np.float32)
    for b in range(B):
        cp = ocp[b].astype(np.float32).reshape(128, NC, D).transpose(1, 0, 2).reshape(Lv, D)
        ch = och[b].astype(np.float32).reshape(128, NC, D).transpose(1, 0, 2).reshape(Lv, D)
        # An empty attended side makes the device denominator 0 (NaN out);
        # the reference defines that case as all-zeros — keep the zeros.
        if n_h[b] > 0:
            out_p[b, idx_p[b]] = cp[: n_p[b]]
        if n_p[b] > 0:
            out_h[b, idx_h[b]] = ch[: n_h[b]]
    return (out_p, out_h), res


def kernel(premise_batch, premise_mask, hypothesis_batch, hypothesis_mask):
    outs, _ = run(premise_batch, premise_mask, hypothesis_batch, hypothesis_mask)
    return outs



# revision 47
# speedup vs baseline: 1.0036x; 1.0036x over previous
"""Fused bidirectional (ESIM) attention kernel for Trainium2 (Bass/Tile).

Problem: B=16, Lp=Lh=2048, D=256 fp32.
  sim = P @ H^T / sqrt(D)
  attended_premises   = masked_softmax(sim,   hm) @ H * pm
  attended_hypotheses = masked_softmax(sim^T, pm) @ P * hm

Key identities / tricks:
  - softmax(scores*mask)*mask / (sum + EPS) reduces to
    out_j = e_j*m_j / sum_k e_k*m_k with e = exp(scores).
  - Masks are dense 0/1 with p=0.5: the host COMPACTS each sequence to its
    valid rows (padded to Lv = max valid count rounded up to 128). All
    device matmuls run on ~Lv=1152 instead of 2048 rows per side (~3.2x
    less PE work; PE is the bottleneck). Outputs scatter back on the host.
  - bf16 operands (fp32 PSUM accumulation): same PE rate as float32r but
    half the DMA traffic; rel-err ~5e-3, inside the 2e-2 gate.
  - Scores are computed ONCE (orientation [h, p]). The second direction's
    E^T comes from dma_start_transpose (DMA xbar, SBUF->SBUF, block-major
    fold [128, G*128] -> [128, G, 128] verified on HW), saving 25% of PE
    work vs recomputing the scores transposed. E chunks are grouped 3 per
    tile so one transpose instruction covers 3 chunks (the per-instruction
    queue cost is what hurts).
  - Both directions' softmax denominators ride the weighted-sum matmuls
    via two valid-indicator columns LEADING the compacted K-side rows
    (N=258, layout [1 1 | data]); no separate reduction anywhere. Leading
    placement lets the final tile's split accumulation normalize early.
  - Outputs are written bf16 (computed in fp32, rounded on store; host
    converts back) — halves output DMA and leaves the sync ring ~10us of
    slack (it is otherwise ~saturated at its ~196 GB/s effective rate).
  - DMA queue discipline: the SP queue pays ~5ns per descriptor and each
    dma_start costs ~0.6us of sequencer issue (128 descriptors), so loads
    are 2 fat DMAs per batch and outputs are staged in SBUF and written as
    3-chunk contiguous DMAs. Fine-grained region-split loads backfire:
    they inflate the NEFF semaphore-init phase (~+0.4us per extra region
    DMA when many regions interleave).
  - A PE warmup burst of dummy matmuls runs during the ~8us DMA/queue
    startup so real matmuls start at the HAM-warm 2.4 GHz clock (the ramp
    needs ~5.6us of sustained PE activity and sags within ~2us of idle).
  - The last weighted-sum tile's serial recip->mul->dma tail (~1.7us) is
    shortened by accumulating it in two PSUM tiles (dep tracking is tile-
    granular): denominator + 160 data cols first, so the reciprocal and
    first chunk's normalize+DMA hide under the 96-col remainder's matmuls;
    final-direction drains alternate the scalar/sync queues.

Sharding: data-parallel over batch, 2 batches per core on 8 cores.

Per batch (Lv padded valid length, NC = Lv/128 chunks, GC = NC/3 groups):
  A1: V1[h,p] per 128-row h-chunk; exp -> E1 chunk (bf16, grouped 3/tile);
      per group one dma-transpose into the E2 buffer [128, 27, 128].
  B1: per p-tile: acc[p, :258] = sum_h E1ᵀ @ [1 1 | Hc]; out = acc[:, 2:]
      * 1/acc[:, 0] into a staging tile, 3-chunk DMAs out.
  B2: same with E2 group slices and [1 1 | Pc].
Emission order A1(0) B1(0) A1(1) B2(0) B1(1) B2(1) keeps the PE FIFO from
head-of-line blocking on batch-0 transposes.
"""

import numpy as np
import ml_dtypes

import concourse.mybir as mybir
import concourse.tile as tile
from concourse import bacc
from concourse.bass_utils import run_bass_kernel_spmd

F32 = mybir.dt.float32
BF16 = mybir.dt.bfloat16
EXP = mybir.ActivationFunctionType.Exp
BF16NP = ml_dtypes.bfloat16

B, L, D = 16, 2048, 256
NCORES = 8
BPC = B // NCORES      # batches per core
DC = D // 128          # 2 contraction chunks of 128 for the score matmuls
NAUG = D + 2           # compacted K-side rows + two valid-indicator columns
GRP = 3                # E chunks per transpose group
SCALE = 1.0 / np.sqrt(np.float32(D)).astype(np.float32)


def PSW(Lv):
    """Score-PSUM tile width, rounded to a whole PSUM bank multiple."""
    return ((Lv * 4 + 2047) // 2048) * 512


def _scores(nc, ldA, Lv, Nq, psv, ep, e2p, tag, soft=False, pac=None):
    """Single score pass: V1[h,p] per h-chunk, exp into grouped E tiles,
    one dma-transpose per group into the E2 chunk-folded layout.

    Score matmuls only cover query columns [0, Nq) (the max valid premise
    count): E columns beyond hold exp(stale PSUM), which is bounded (PSUM
    score banks only ever contain zeros from warmup or old score values),
    and they only feed dropped output partitions / zeroed contractions.

    Returns (E1 group tiles, E2 group tiles)."""
    NC = Lv // 128
    GC = NC // GRP
    psw = PSW(Lv)
    la0, la1 = ldA
    xth = [la0[:, 0:Lv], la1[:, 0:Lv]]
    xtp = [la0[:, Lv : 2 * Lv], la1[:, Lv : 2 * Lv]]
    e1g, e2g = [], []
    for g in range(GC):
        e1g.append(ep.tile([128, GRP, Lv], BF16, tag=f"E{g}", name=f"E{tag}_{g}"))
        e2g.append(
            e2p.tile([128, GRP * NC, 128], BF16, tag=f"T{g}", name=f"T{tag}_{g}")
        )
    def mm_dc(ps, kc, dc):
        for off in range(0, Nq, 512):
            w = min(512, Nq - off)
            nc.tensor.matmul(
                ps[:, off : off + w],
                lhsT=xth[dc][:, kc * 128 : (kc + 1) * 128],
                rhs=xtp[dc][:, off : off + w],
                start=(dc == 0),
                stop=(dc == DC - 1),
            )

    def finish(ps, kc):
        g, r = divmod(kc, GRP)
        nc.scalar.activation(e1g[g][:, r, :], ps[:, 0:Lv], EXP, scale=float(SCALE))
        if r == GRP - 1:
            nc.sync.dma_start_transpose(e2g[g][:, :, :], e1g[g][:, :, :])

    kc0 = 0
    if soft and NC >= 2 and DC == 2:
        # Kernel-start softening: the dc=1 operands (second load half) land
        # ~2us after dc=0's. Run BOTH first chunks' dc=0 sweeps (two open
        # PSUM accumulations = both psv buffers) before touching dc=1 so
        # the PE has work while that transfer finishes.
        ps0 = psv.tile([128, psw], F32, tag="v", name=f"v{tag}0")
        ps1 = psv.tile([128, psw], F32, tag="v", name=f"v{tag}1")
        mm_dc(ps0, 0, 0)
        mm_dc(ps1, 1, 0)
        # Runway extension: both psv buffers are open, but the wsum
        # accumulator banks (pac) are idle until ~25us — park kc2/kc3's
        # dc=0 first-512 columns there so the PE has ~0.7us more dc=0 work
        # before it must block on the dc=1 load. Same total sweep count;
        # those chunks' exp later reads the pac bank for [0:512) and the
        # psv tile for the rest.
        w0 = 512
        pcs = []
        if pac is not None and NC >= 4 and Nq > w0:
            for i in range(2):
                kci = 2 + i
                pc = pac.tile([128, 512], F32, tag="acc", name=f"pc{tag}_{kci}")
                nc.tensor.matmul(
                    pc[:, 0:w0],
                    lhsT=xth[0][:, kci * 128 : (kci + 1) * 128],
                    rhs=xtp[0][:, 0:w0],
                    start=True,
                    stop=False,
                )
                pcs.append(pc)
        mm_dc(ps0, 0, 1)
        finish(ps0, 0)
        mm_dc(ps1, 1, 1)
        finish(ps1, 1)
        kc0 = 2
        for i, pc in enumerate(pcs):
            kci = 2 + i
            nc.tensor.matmul(
                pc[:, 0:w0],
                lhsT=xth[1][:, kci * 128 : (kci + 1) * 128],
                rhs=xtp[1][:, 0:w0],
                start=False,
                stop=True,
            )
            ps = psv.tile([128, psw], F32, tag="v", name=f"v{tag}{kci}")
            for dc in range(DC):
                for off in range(w0, Nq, 512):
                    w = min(512, Nq - off)
                    nc.tensor.matmul(
                        ps[:, off : off + w],
                        lhsT=xth[dc][:, kci * 128 : (kci + 1) * 128],
                        rhs=xtp[dc][:, off : off + w],
                        start=(dc == 0),
                        stop=(dc == DC - 1),
                    )
            g, r = divmod(kci, GRP)
            nc.scalar.activation(
                e1g[g][:, r, 0:w0], pc[:, 0:w0], EXP, scale=float(SCALE)
            )
            nc.scalar.activation(
                e1g[g][:, r, w0:Lv], ps[:, w0:Lv], EXP, scale=float(SCALE)
            )
            if r == GRP - 1:
                nc.sync.dma_start_transpose(e2g[g][:, :, :], e1g[g][:, :, :])
            kc0 = kci + 1
    for kc in range(kc0, NC):
        ps = psv.tile([128, psw], F32, tag="v", name=f"v{tag}{kc}")
        for dc in range(DC):
            mm_dc(ps, kc, dc)
        finish(ps, kc)
    return e1g, e2g


def _wsum(nc, lhs_of, xa, out_dram, Lv, pac, den, outp, tag, final=False,
          soft=False, psv=None):
    """One direction's weighted sum + normalize + staged output DMAs.

    lhs_of(qt, kc) -> [128, 128] bf16 lhsT slice (contraction chunk kc for
    query tile qt). xa: [128, NC*NAUG] K-side rows with valid columns.
    Outputs stage in SBUF and leave as 3-chunk DMAs on the Sync queue;
    final=True instead streams per-tile from the (by then idle) ACT hwdge
    queue so the kernel tail is one 128-row transfer.
    soft=True interleaves the first two query tiles' accumulations (both
    PSUM accumulator buffers open) so their last-group matmuls don't catch
    up with the score pass's final exp (whole-tile dependency)."""
    NC = Lv // 128
    accs = {}

    def mm_run(qt, k0, k1, c0=0, c1=NAUG):
        for kc in range(k0, k1):
            nc.tensor.matmul(
                accs[qt][:, c0:c1],
                lhsT=lhs_of(qt, kc),
                rhs=xa[:, kc * NAUG + c0 : kc * NAUG + c1],
                start=(kc == 0),
                stop=(kc == NC - 1),
            )

    def finish(qt, stage, qr):
        # Augmented-row layout: cols [0:2) valid indicators (denominator),
        # data at [2:NAUG).
        r = den.tile([128, 1], F32, tag="rec", name=f"rec{tag}_{qt}")
        nc.vector.reciprocal(r[:], accs[qt][:, 0:1])
        nc.vector.tensor_scalar_mul(
            stage[:, qr * D : (qr + 1) * D], accs[qt][:, 2 : 2 + D], r[:]
        )
        if final:
            # Alternate per-tile drain DMAs over two queues: a dma_start
            # costs ~0.6us of sequencer issue time (128 descriptors), so on
            # one queue the last tile's write would serialize behind eight
            # earlier issues.
            eng = (nc.scalar, nc.sync)[qt % 2]
            eng.dma_start(
                out=out_dram[:, qt * D : (qt + 1) * D],
                in_=stage[:, qr * D : (qr + 1) * D],
            )

    def finish_tail(qt, stage, qr, psv):
        # The very last tile gates the kernel end with a serial
        # recip->mul->dma chain (~1.7us). Its accumulation runs as two
        # column groups in SEPARATE PSUM tiles (dep tracking is tile-
        # granular, so one shared tile would serialize the reciprocal
        # behind both chains) — the first group holds the denominator
        # plus most of the data, so the reciprocal and the first chunk's
        # normalize+DMA hide under the second group's matmuls; only a
        # 96-col remainder drains after the last matmul.
        da = 160
        ca = 2 + da
        accb = pac.tile([128, 512], F32, tag="acc", name=f"accb{tag}")
        mm_run(qt, 0, NC, 0, ca)
        for kc in range(NC):
            nc.tensor.matmul(
                accb[:, 0 : NAUG - ca],
                lhsT=lhs_of(qt, kc),
                rhs=xa[:, kc * NAUG + ca : (kc + 1) * NAUG],
                start=(kc == 0),
                stop=(kc == NC - 1),
            )
        r = den.tile([128, 1], F32, tag="rec", name=f"rec{tag}_{qt}")
        nc.vector.reciprocal(r[:], accs[qt][:, 0:1])
        nc.vector.tensor_scalar_mul(
            stage[:, qr * D : qr * D + da], accs[qt][:, 2:ca], r[:]
        )
        nc.scalar.dma_start(
            out=out_dram[:, qt * D : qt * D + da],
            in_=stage[:, qr * D : qr * D + da],
        )
        nc.vector.tensor_scalar_mul(
            stage[:, qr * D + da : (qr + 1) * D], accb[:, 0 : NAUG - ca], r[:]
        )
        nc.sync.dma_start(
            out=out_dram[:, qt * D + da : (qt + 1) * D],
            in_=stage[:, qr * D + da : (qr + 1) * D],
        )

    for g in range(NC // GRP):
        stage = outp.tile(
            [128, GRP * D], BF16, tag=f"st{tag[0]}{g}", name=f"st{tag}_{g}"
        )
        for qr in range(GRP):
            qt = g * GRP + qr
            if soft and NC > GRP and g == 0 and qr in (0, 1):
                if qr == 0:
                    # qt0 up to the last chunk group, then qt1 likewise,
                    # then both tails — by which time the final exp landed.
                    accs[0] = pac.tile(
                        [128, 512], F32, tag="acc", name=f"acc{tag}_0"
                    )
                    accs[1] = pac.tile(
                        [128, 512], F32, tag="acc", name=f"acc{tag}_1"
                    )
                    mm_run(0, 0, NC - GRP)
                    mm_run(1, 0, NC - GRP)
                    mm_run(0, NC - GRP, NC)
                    finish(0, stage, 0)
                else:
                    mm_run(1, NC - GRP, NC)
                    finish(1, stage, 1)
                continue
            accs[qt] = pac.tile([128, 512], F32, tag="acc", name=f"acc{tag}_{qt}")
            if final and qt == NC - 1:
                finish_tail(qt, stage, qr, psv)
            else:
                mm_run(qt, 0, NC)
                finish(qt, stage, qr)
        if not final:
            nc.sync.dma_start(
                out=out_dram[:, g * GRP * D : (g + 1) * GRP * D],
                in_=stage[:],
            )


def build_program(Lv, Nq, bpc=BPC):
    NC = Lv // 128
    assert NC % GRP == 0
    # The tuned pipeline (double buffering everywhere) is sized for the
    # expected ~50% masks (Lv=1152). Denser masks still fit on-device up to
    # Lv=1920 with single buffering; beyond that run() falls back to host.
    sb = 2 if Lv <= 1152 else 1
    pb = 2 if 2 * (PSW(Lv) // 512) + 2 <= 8 else 1
    nc = bacc.Bacc("TRN2", target_bir_lowering=False, debug=False, num_devices=NCORES)
    # ldA: the four d-major score operands [xth0|xtp0|xth1|xtp1].
    # ldB: the two augmented K-side row blocks [xAh|xAp].
    ldA_t = nc.dram_tensor("ldA", [bpc, 128, 4 * Lv], BF16, kind="ExternalInput").ap()
    ldB_t = nc.dram_tensor(
        "ldB", [bpc, 128, 2 * NC * NAUG], BF16, kind="ExternalInput"
    ).ap()
    out_p = nc.dram_tensor("out_prem", [bpc, 128, NC * D], BF16, kind="ExternalOutput").ap()
    out_h = nc.dram_tensor("out_hyp", [bpc, 128, NC * D], BF16, kind="ExternalOutput").ap()

    with tile.TileContext(nc) as tc:
        with (
            tc.tile_pool(name="tp", bufs=sb) as tp,
            tc.tile_pool(name="ep", bufs=sb) as ep,
            tc.tile_pool(name="e2p", bufs=sb) as e2p,
            tc.tile_pool(name="outp", bufs=sb) as outp,
            tc.tile_pool(name="psv", bufs=pb, space="PSUM") as psv,
            tc.tile_pool(name="pac", bufs=2, space="PSUM") as pac,
            tc.tile_pool(name="den", bufs=4) as den,
        ):
            # PE warmup: queue bring-up + the first operand DMA keep the PE
            # data-starved until ~10.7us; dummy matmuls meanwhile ramp the
            # HAM clock so real matmuls start nearer 2.4 GHz. The dummies
            # also sweep every (buffer, bank) region of the score-PSUM
            # pool, zeroing whatever a previous NEFF left there (exp of a
            # stale inf in the skipped pad columns would ride the transpose
            # into direction-2 lhsT rows and NaN valid outputs via inf*0).
            wuw = tp.tile([128, 128], BF16, tag="wuw", name="wuw")
            nc.vector.memset(wuw[:], 0)
            wur = tp.tile([128, 512], BF16, tag="wur", name="wur")
            nc.vector.memset(wur[:], 0)
            nreg = PSW(Lv) // 512
            for i in range(9):
                pswu = psv.tile([128, PSW(Lv)], F32, tag="v", name=f"wu{i}")
                off = ((i // 2) % nreg) * 512
                nc.tensor.matmul(
                    pswu[:, off : off + 512], lhsT=wuw[:], rhs=wur[:],
                    start=True, stop=True,
                )

            def loads(b, fine=False):
                # Split the score operands in two so the dc=0 pair lands
                # ~1.5us earlier than one fat transfer would. For the head-
                # critical first batch, la0 splits further (lhsT half, then
                # the rhs in sweep-sized pieces) so the first score sweep
                # starts ~0.5us before the full la0 would have landed.
                # Aggressive fine-splitting backfires: many-region DMA
                # patterns inflate the NEFF's semaphore-init phase
                # (measured +2.9us for 7 extra region DMAs).
                la0 = tp.tile([128, 2 * Lv], BF16, tag="ldA0", name=f"ldA0_{b}")
                if fine and Lv > 512:
                    nc.sync.dma_start(out=la0[:, 0:Lv], in_=ldA_t[b, :, 0:Lv])
                    nc.sync.dma_start(
                        out=la0[:, Lv : Lv + 512], in_=ldA_t[b, :, Lv : Lv + 512]
                    )
                    nc.sync.dma_start(
                        out=la0[:, Lv + 512 : 2 * Lv],
                        in_=ldA_t[b, :, Lv + 512 : 2 * Lv],
                    )
                else:
                    nc.sync.dma_start(out=la0[:], in_=ldA_t[b, :, 0 : 2 * Lv])
                la1 = tp.tile([128, 2 * Lv], BF16, tag="ldA1", name=f"ldA1_{b}")
                nc.sync.dma_start(out=la1[:], in_=ldA_t[b, :, 2 * Lv : 4 * Lv])
                lb = tp.tile([128, 2 * NC * NAUG], BF16, tag="ldB", name=f"ldB{b}")
                nc.sync.dma_start(out=lb[:], in_=ldB_t[b])
                return (la0, la1), lb

            def d1_lhs(e1g):
                def f(qt, kc):
                    return e1g[kc // GRP][:, kc % GRP, qt * 128 : (qt + 1) * 128]
                return f

            def d2_lhs(e2g):
                def f(qt, kc):
                    # e2 group tile j-index = (h-chunk within group)*NC + pc
                    return e2g[qt // GRP][:, (qt % GRP) * NC + kc, :]
                return f

            st = [None] * bpc
            eg = [None] * bpc
            st[0] = loads(0)
            eg[0] = _scores(
                nc, st[0][0], Lv, Nq, psv, ep, e2p, "0", soft=True, pac=pac
            )
            _wsum(
                nc, d1_lhs(eg[0][0]), st[0][1][:, 0 : NC * NAUG], out_p[0],
                Lv, pac, den, outp, "p0", soft=True,
            )
            for b in range(bpc):
                if b + 1 < bpc:
                    st[b + 1] = loads(b + 1)
                    eg[b + 1] = _scores(
                        nc, st[b + 1][0], Lv, Nq, psv, ep, e2p, f"{b+1}"
                    )
                _wsum(
                    nc, d2_lhs(eg[b][1]), st[b][1][:, NC * NAUG :], out_h[b],
                    Lv, pac, den, outp, f"h{b}", final=(b == bpc - 1), psv=psv,
                )
                if b + 1 < bpc:
                    _wsum(
                        nc, d1_lhs(eg[b + 1][0]), st[b + 1][1][:, 0 : NC * NAUG],
                        out_p[b + 1], Lv, pac, den, outp, f"p{b+1}", soft=True,
                    )
    nc.compile()
    return nc


_PROGRAMS = {}


def _get_program(Lv, Nq):
    if (Lv, Nq) not in _PROGRAMS:
        _PROGRAMS[(Lv, Nq)] = build_program(Lv, Nq)
    return _PROGRAMS[(Lv, Nq)]


def _prep_side(x, idx, n, Lv):
    """Compact one (batch, side) to its valid rows.

    Returns (xT [128, 2*Lv] d-major halves, xA [128, NC*NAUG] chunk-row
    layout with two LEADING valid-indicator columns — the denominator rides
    in front so the final tile's split accumulation can normalize its first
    half early)."""
    NC = Lv // 128
    xc = np.zeros((Lv, NAUG), np.float32)
    xc[:n, 0:2] = 1.0
    xc[:n, 2:] = x[idx]
    t = xc[:, 2:].reshape(Lv, DC, 128).transpose(1, 2, 0)  # [dc, d, row]
    xA = np.ascontiguousarray(
        xc.reshape(NC, 128, NAUG).transpose(1, 0, 2).reshape(128, NC * NAUG)
    )
    return t, xA


def _host_fallback(pb, hb, pm, hm):
    """Exact reference semantics in numpy for inputs the device path can't
    hold (mask density far above the expected ~50%)."""
    def msoft(s, m):
        s = s * m
        s = s - s.max(axis=-1, keepdims=True)
        e = np.exp(s) * m
        return e / (e.sum(axis=-1, keepdims=True) + 1e-13)

    out_p = np.empty(pb.shape, np.float32)
    out_h = np.empty(hb.shape, np.float32)
    scale = 1.0 / np.sqrt(pb.shape[-1])
    for b in range(pb.shape[0]):
        sim = (pb[b].astype(np.float64) @ hb[b].T.astype(np.float64)) * scale
        pmb = pm[b].astype(np.float64)
        hmb = hm[b].astype(np.float64)
        out_p[b] = (msoft(sim, hmb[None, :]) @ hb[b].astype(np.float64)) * pmb[:, None]
        out_h[b] = (msoft(sim.T, pmb[None, :]) @ pb[b].astype(np.float64)) * hmb[:, None]
    return out_p, out_h


def run(premise_batch, premise_mask, hypothesis_batch, hypothesis_mask, trace=False):
    pb = np.asarray(premise_batch, dtype=np.float32)
    hb = np.asarray(hypothesis_batch, dtype=np.float32)
    pm = np.asarray(premise_mask)
    hm = np.asarray(hypothesis_mask)

    idx_p = [np.flatnonzero(pm[b]) for b in range(B)]
    idx_h = [np.flatnonzero(hm[b]) for b in range(B)]
    n_p = [len(i) for i in idx_p]
    n_h = [len(i) for i in idx_h]
    n_max = max(max(n_p), max(n_h), 128)
    Lv = ((n_max + 383) // 384) * 384  # multiple of 384 so NC % GRP == 0
    if Lv > 1920:
        # Near-dense masks don't fit the compacted on-device pipeline;
        # fall back to a correct host computation (not the perf path).
        return _host_fallback(pb, hb, pm, hm), None
    Nq = min(((max(max(n_p), 1) + 1) // 2) * 2, Lv)  # score query extent
    nc = _get_program(Lv, Nq)

    NC = Lv // 128
    ldA = np.empty((B, 128, 4 * Lv), BF16NP)
    ldB = np.empty((B, 128, 2 * NC * NAUG), BF16NP)
    for b in range(B):
        tp_, xAp = _prep_side(pb[b], idx_p[b], n_p[b], Lv)
        th_, xAh = _prep_side(hb[b], idx_h[b], n_h[b], Lv)
        ldA[b, :, 0 * Lv : 1 * Lv] = th_[0]
        ldA[b, :, 1 * Lv : 2 * Lv] = tp_[0]
        ldA[b, :, 2 * Lv : 3 * Lv] = th_[1]
        ldA[b, :, 3 * Lv : 4 * Lv] = tp_[1]
        ldB[b, :, 0 : NC * NAUG] = xAh
        ldB[b, :, NC * NAUG :] = xAp

    in_maps = []
    for c in range(NCORES):
        s = slice(c * BPC, (c + 1) * BPC)
        in_maps.append({"ldA": ldA[s], "ldB": ldB[s]})
    res = None
    for attempt in range(3):
        try:
            res = run_bass_kernel_spmd(nc, in_maps, list(range(NCORES)), trace=trace)
            break
        except Exception:
            # Transient device wedges (NRT_EXEC_UNIT_UNRECOVERABLE etc.)
            # usually clear on re-execution.
            if attempt == 2:
                raise
    ocp = np.concatenate([res.results[c]["out_prem"] for c in range(NCORES)], axis=0)
    och = np.concatenate([res.results[c]["out_hyp"] for c in range(NCORES)], axis=0)
    out_p = np.zeros((B, L, D), np.float32)
    out_h = np.zeros((B, L, D), np.float32)
    for b in range(B):
        cp = ocp[b].astype(np.float32).reshape(128, NC, D).transpose(1, 0, 2).reshape(Lv, D)
        ch = och[b].astype(np.float32).reshape(128, NC, D).transpose(1, 0, 2).reshape(Lv, D)
        # An empty attended side makes the device denominator 0 (NaN out);
        # the reference defines that case as all-zeros — keep the zeros.
        if n_h[b] > 0:
            out_p[b, idx_p[b]] = cp[: n_p[b]]
        if n_p[b] > 0:
            out_h[b, idx_h[b]] = ch[: n_h[b]]
    return (out_p, out_h), res


def kernel(premise_batch, premise_mask, hypothesis_batch, hypothesis_mask):
    outs, _ = run(premise_batch, premise_mask, hypothesis_batch, hypothesis_mask)
    return outs

